# revision 2
# baseline (speedup 1.0000x reference)
"""Trainium2 Bass kernel v3 for GQA attention forward (B=2, S=2048, D=2048,
16 q-heads / 4 kv-heads, head_dim=128, RoPE, causal).

Sharding: 8 cores = 2 (batch) x 4 (kv-head groups); host sums the 4
row-parallel output-projection partials per batch.

v3: single chunk-fused pipeline. For each 512-row chunk c:
  A(c)  project x[c] -> q,k,v + RoPE (PE+DVE), Q/K transposes on the DMA
        xbar (split across both HWDGE queues)
  B(c)  causal attention for the 4 heads of this group against k/v[0..c]
  C(c-1) output projection of the previous chunk, injected into B(c)'s
        instruction stream so PE never waits on the Activation engine
All PSUM lives in 3 pools (8 banks total); phase-A accumulators share the
scratch ring with score tiles, ps_b and the projection outputs.
"""

import sys

if "/opt/trn_rl_repo" not in sys.path:
    sys.path.insert(0, "/opt/trn_rl_repo")

import numpy as np
import ml_dtypes

import concourse.bass as bass
import concourse.tile as tile
from concourse import mybir

F32 = mybir.dt.float32
F32R = mybir.dt.float32r
BF16 = mybir.dt.bfloat16

B, S, DIM = 2, 2048, 2048
N_HEADS, N_KV_HEADS, HEAD_DIM = 16, 4, 128
N_GROUPS = N_KV_HEADS
HQ = N_HEADS // N_KV_HEADS
NEG = -1e30
VW = 132                       # v row: 128 v cols + ones col + pad


def build_attention_core(nc, S=S, D=DIM, HQ=HQ, HD=HEAD_DIM, CHUNK=512):
    n_st = S // 128
    n_dt = D // 128
    n_ch = S // CHUNK
    kpc = CHUNK // 128
    n_dc = D // CHUNK
    IQ = HQ * HD

    x_d = nc.dram_tensor("xT", [S // 512, 128, D // 128, 512], BF16, kind="ExternalInput")
    wqT_d = nc.dram_tensor("wqT", [128, D // 128, IQ], BF16, kind="ExternalInput")
    wkvT_d = nc.dram_tensor("wkvT", [128, D // 128, 2 * HD], BF16, kind="ExternalInput")
    woT_d = nc.dram_tensor("woT", [128, IQ // 128, D], BF16, kind="ExternalInput")
    t1_d = nc.dram_tensor("t1", [128, S // 128, HD], BF16, kind="ExternalInput")
    t2_d = nc.dram_tensor("t2", [128, S // 128, HD], BF16, kind="ExternalInput")
    masks_d = nc.dram_tensor("masks", [128, kpc, CHUNK], BF16, kind="ExternalInput")
    out_d = nc.dram_tensor("out_partial", [S, D], BF16, kind="ExternalOutput")

    scale = float(HD) ** -0.5

    with tile.TileContext(nc) as tc:
        with (
            tc.tile_pool(name="persist", bufs=1) as persist,
            tc.tile_pool(name="weights", bufs=1) as weights,
            tc.tile_pool(name="xt", bufs=3) as xt_pool,
            tc.tile_pool(name="rope", bufs=2) as rope_pool,
            tc.tile_pool(name="qrot", bufs=3) as qrot_pool,
            tc.tile_pool(name="exps", bufs=20) as exp_pool,
            tc.tile_pool(name="onorm", bufs=3) as on_pool,
            tc.tile_pool(name="recips", bufs=8) as rec_pool,
            tc.tile_pool(name="outsb", bufs=2) as outsb_pool,
            tc.tile_pool(name="ps_sc", bufs=4, space="PSUM") as scratch,
            tc.tile_pool(name="ps_oo", bufs=1, space="PSUM") as pso_pool,
        ):
            qkT_sb = persist.tile([128, HQ + 1, S], BF16)  # [e, h|k, s]
            qT_sb = qkT_sb[:, 0:HQ, :]
            kT_sb = qkT_sb[:, HQ, :]
            v_sb = persist.tile([128, n_st, VW], BF16)  # [s_in_tile, s_tile, e|1]
            oT_sb = persist.tile([128, HQ, S], BF16)    # [e, h, s]

            # ---- ALL DMA on the sync queue: a DMA-xbar transpose in
            # flight concurrently with a transfer on the other HWDGE queue
            # corrupts data, so everything is FIFO-serialized on one queue.
            gq = n_dt // 4

            xt_tiles = [None] * n_ch

            def load_xt(c):
                xt_t = xt_pool.tile([128, n_dt, CHUNK], BF16, tag="xt",
                                    name=f"xt{c}")
                nc.sync.dma_start(out=xt_t, in_=x_d[c])
                xt_tiles[c] = xt_t

            # ones column for the fused denominator (v copies fill 0:HD)
            nc.vector.memset(v_sb, 1.0)
            warm = weights.tile([128, 1], BF16, name="warm")
            load_xt(0)
            wq_g = []
            wkv_g = []
            for g in range(4):
                wqg = weights.tile([128, gq, IQ], BF16, tag=f"wq{g}",
                                   name=f"wq{g}")
                nc.sync.dma_start(out=wqg, in_=wqT_d[:, g * gq:(g + 1) * gq, :])
                wq_g.append(wqg)
                wkvg = weights.tile([128, gq, 2 * HD], BF16, tag=f"wkv{g}",
                                    name=f"wkv{g}")
                nc.sync.dma_start(out=wkvg, in_=wkvT_d[:, g * gq:(g + 1) * gq, :])
                wkv_g.append(wkvg)
            t1_sb = weights.tile([128, n_st, HD], BF16)
            nc.sync.dma_start(out=t1_sb, in_=t1_d[:])
            t2_sb = weights.tile([128, n_st, HD], BF16)
            nc.sync.dma_start(out=t2_sb, in_=t2_d[:])
            masks_sb = weights.tile([128, kpc, CHUNK], BF16)
            nc.sync.dma_start(out=masks_sb, in_=masks_d[:])
            load_xt(1)
            load_xt(2)
            woT_sb = weights.tile([128, IQ // 128, D], BF16)

            # ---------------- helpers --------------------------------------
            def emit_A_stile(c, stl):
                """Project one 128-row tile: q,k,v + RoPE + xbar transposes."""
                st = c * kpc + stl
                xt_c = xt_tiles[c]
                ps_q = scratch.tile([128, CHUNK], F32, tag="s", name="ps_q")
                ps_kv = scratch.tile([128, CHUNK], F32, tag="s", name="ps_kv")
                sl = slice(stl * 128, (stl + 1) * 128)
                for dt_ in range(n_dt):
                    nc.tensor.matmul(
                        ps_q, xt_c[:, dt_, sl], wq_g[dt_ // gq][:, dt_ % gq, :],
                        start=(dt_ == 0), stop=(dt_ == n_dt - 1),
                    )
                    nc.tensor.matmul(
                        ps_kv[:, 0:2 * HD], xt_c[:, dt_, sl],
                        wkv_g[dt_ // gq][:, dt_ % gq, :],
                        start=(dt_ == 0), stop=(dt_ == n_dt - 1),
                    )
                # RoPE on all q heads at once
                t1s = t1_sb[:, st, :]
                t2s = t2_sb[:, st, :]
                t1b = bass.AP(tensor=t1s.tensor, offset=t1s.offset,
                              ap=[t1s.ap[0], [0, HQ], t1s.ap[1]])
                t2b = bass.AP(tensor=t2s.tensor, offset=t2s.offset,
                              ap=[t2s.ap[0], [0, HQ], t2s.ap[1]])
                ps_qv = ps_q.rearrange("p (h e) -> p h e", h=HQ)
                t1m = rope_pool.tile([128, HQ, HD], F32, tag="t1m")
                nc.vector.tensor_mul(t1m, ps_qv, t1b)
                t2m = rope_pool.tile([128, HQ, HD], F32, tag="t2m")
                nc.vector.tensor_mul(
                    t2m[:, :, 0:64], ps_qv[:, :, 64:128], t2b[:, :, 0:64]
                )
                nc.vector.tensor_mul(
                    t2m[:, :, 64:128], ps_qv[:, :, 0:64], t2b[:, :, 64:128]
                )
                q_rot = qrot_pool.tile([128, (HQ + 1) * HD], BF16, tag="qrot")
                nc.vector.tensor_add(
                    q_rot[:, 0:HQ * HD].rearrange("p (h e) -> p h e", h=HQ),
                    t1m, t2m
                )
                t1mk = rope_pool.tile([128, HD], F32, tag="t1mk")
                nc.vector.tensor_mul(t1mk, ps_kv[:, 0:HD], t1s)
                t2mk = rope_pool.tile([128, HD], F32, tag="t2mk")
                nc.vector.tensor_mul(
                    t2mk[:, 0:64], ps_kv[:, 64:128], t2s[0:128, 0:64]
                )
                nc.vector.tensor_mul(
                    t2mk[:, 64:128], ps_kv[:, 0:64], t2s[0:128, 64:128]
                )
                k_rot = q_rot[:, HQ * HD:(HQ + 1) * HD]
                nc.vector.tensor_add(k_rot, t1mk, t2mk)

                # v: straight copy (natural [s, e] layout), cast to bf16
                nc.scalar.copy(v_sb[:, st, 0:HD], ps_kv[:, HD:2 * HD])

                # transposes on the DMA xbar (sync queue only — concurrent
                # xbar transposes from two HWDGE queues corrupt data).
                # One [128,512] transpose covers all 4 q heads: the xbar maps
                # input column h*128+e to output [e, h] of a 3D out AP.
                s128 = slice(st * 128, (st + 1) * 128)
                nc.sync.dma_start(out=qkT_sb[:, :, s128], in_=q_rot,
                                  transpose=True)

            def emit_C(c):
                """Output projection for chunk c (16 (st,dc) units)."""
                for st in range(c * kpc, (c + 1) * kpc):
                    ob = outsb_pool.tile([128, D], BF16, tag="ob", name="ob")
                    for dc in range(n_dc):
                        ps_d = scratch.tile([128, CHUNK], F32, tag="s",
                                            name="ps_d")
                        for it in range(HQ):
                            nc.tensor.matmul(
                                ps_d,
                                oT_sb[:, it, st * 128:(st + 1) * 128],
                                woT_sb[:, it, dc * CHUNK:(dc + 1) * CHUNK],
                                start=(it == 0), stop=(it == HQ - 1),
                            )
                        nc.vector.tensor_copy(
                            ob[:, dc * CHUNK:(dc + 1) * CHUNK], ps_d
                        )
                        if dc % 2 == 1:
                            nc.sync.dma_start(
                                out=out_d[st * 128:(st + 1) * 128,
                                          (dc - 1) * CHUNK:(dc + 1) * CHUNK],
                                in_=ob[:, (dc - 1) * CHUNK:(dc + 1) * CHUNK],
                            )

            # ---------------- fused chunk pipeline -------------------------
            # Emission order: A0 A1 B0 A2 B1 A3 B2 B3 — one-chunk lookahead
            # so chunk c's xbar transposes drain during A(c+1).
            def emit_A_chunk(c):
                for stl in range(kpc):
                    emit_A_stile(c, stl)

            emit_A_chunk(0)
            # preload the exp table set during phase-A slack
            nc.scalar.activation(warm, t1_sb[:, 0, 0:1],
                                 mybir.ActivationFunctionType.Exp)
            emit_A_chunk(1)
            load_xt(3)
            nc.sync.dma_start(out=woT_sb, in_=woT_d[:])
            for c in range(n_ch):
                # B(c): X-orientation — AV runs lhsT=exp-subtile so one
                # matmul yields both context and denominator (ones col of v)
                dtile = c * kpc
                n_kj = dtile + kpc
                for h in range(HQ):
                    exp_tiles = [None] * n_kj

                    def emit_score(kj, h=h, c=c, dtile=dtile,
                                   exp_tiles=exp_tiles):
                        off = max(0, kj - dtile) * 128
                        w = CHUNK - off
                        ps = scratch.tile([128, CHUNK], F32, tag="s",
                                          name="ps_s")
                        nc.tensor.matmul(
                            ps[:, 0:w],
                            kT_sb[:, kj * 128:(kj + 1) * 128],
                            qT_sb[:, h, c * CHUNK + off:(c + 1) * CHUNK],
                            start=True, stop=True,
                        )
                        if kj >= dtile:  # diagonal: causal mask in place
                            nc.vector.tensor_add(
                                ps[:, 0:w], ps[:, 0:w],
                                masks_sb[:, kj - dtile, off:],
                            )
                        e_t = exp_pool.tile([128, CHUNK], BF16, tag="e",
                                            name="expT")
                        nc.scalar.activation(
                            e_t[:, 0:w], ps[:, 0:w],
                            mybir.ActivationFunctionType.Exp,
                            scale=scale,
                        )
                        exp_tiles[kj] = (e_t, off)

                    sp = [0]

                    def ensure_scores(upto, sp=sp, n_kj=n_kj,
                                      emit_score=emit_score):
                        while sp[0] <= min(upto, n_kj - 1):
                            emit_score(sp[0])
                            sp[0] += 1

                    ensure_scores(2)
                    if h == 2 and c >= 1:
                        emit_C(c - 1)

                    po_tiles = []
                    for qsub in range(kpc):
                        po = pso_pool.tile([128, VW], F32, tag=f"o{qsub}",
                                           name=f"po{qsub}")
                        po_tiles.append(po)
                        last_kj = dtile + qsub
                        for kj in range(0, last_kj + 1):
                            ensure_scores(kj + 3)
                            e_t, off = exp_tiles[kj]
                            q0 = qsub * 128 - off
                            nc.tensor.matmul(
                                po[:, 0:HD + 1],
                                e_t[:, q0:q0 + 128],
                                v_sb[:, kj, 0:HD + 1],
                                start=(kj == 0), stop=(kj == last_kj),
                            )

                    # normalize per-partition (q) and transpose back via xbar
                    o_n4 = on_pool.tile([128, CHUNK], BF16, tag="on",
                                        name="o_n4")
                    for qsub in range(kpc):
                        po = po_tiles[qsub]
                        rc = rec_pool.tile([128, 1], F32, tag="rc", name="rc")
                        nc.vector.reciprocal_approx_fast(rc, po[:, HD:HD + 1])
                        nc.vector.tensor_scalar_mul(
                            o_n4[:, qsub * 128:(qsub + 1) * 128],
                            po[:, 0:HD], rc
                        )
                    nc.sync.dma_start(
                        out=oT_sb[:, h, c * CHUNK:(c + 1) * CHUNK].rearrange(
                            "p (q l) -> p q l", q=kpc),
                        in_=o_n4, transpose=True,
                    )
                if c + 2 < n_ch:
                    emit_A_chunk(c + 2)

            emit_C(n_ch - 1)

    return nc


# ---------------------------------------------------------------------------
# Host-side prep


_ROPE_PERM = np.concatenate([np.arange(0, HEAD_DIM, 2), np.arange(1, HEAD_DIM, 2)])


def _prep_tables(freq_cis, S_=S, HD_=HEAD_DIM):
    """RoPE tables in permuted-half layout: rot = q*t1 + swap(q)*t2."""
    fc = np.asarray(freq_cis, dtype=np.float32)
    A = fc[:, :, 0, 0]
    Bm = fc[:, :, 0, 1]
    C = fc[:, :, 1, 0]
    Dm = fc[:, :, 1, 1]
    t1 = np.concatenate([A, Dm], axis=1).astype(np.float32)  # [S, HD]
    t2 = np.concatenate([Bm, C], axis=1).astype(np.float32)
    return np.ascontiguousarray(t1), np.ascontiguousarray(t2)


def _prep_masks(chunk=512):
    kpc = chunk // 128
    masks = np.zeros((kpc, 128, chunk), dtype=np.float32)
    q = np.arange(chunk)[None, :]
    p = np.arange(128)[:, None]
    for j in range(kpc):
        masks[j] = np.where(q >= j * 128 + p, 0.0, NEG).astype(np.float32)
    return masks


def _perm_head_rows(w):
    """Permute rows within each 128-row head block: evens first, odds second."""
    nh = w.shape[0] // HEAD_DIM
    return np.ascontiguousarray(
        w.reshape(nh, HEAD_DIM, -1)[:, _ROPE_PERM, :].reshape(w.shape)
    )


def _bf16(a):
    return np.ascontiguousarray(a.astype(ml_dtypes.bfloat16))


def _pmajor(a):
    """[T*128, F...] -> [128, T, F...] partition-major layout."""
    t = a.shape[0] // 128
    return np.ascontiguousarray(
        a.reshape(t, 128, *a.shape[1:]).swapaxes(0, 1)
    )


def make_core_inputs(x, freq_cis, wq, wk, wv, wo):
    """Build the 8 per-core input maps."""
    x = np.asarray(x, np.float32)
    wq = np.asarray(wq, np.float32)
    wk = np.asarray(wk, np.float32)
    wv = np.asarray(wv, np.float32)
    wo = np.asarray(wo, np.float32)
    t1, t2 = _prep_tables(freq_cis)
    masks = _prep_masks()
    IQ = HQ * HEAD_DIM

    in_maps = []
    for core in range(8):
        b, g = divmod(core, N_GROUPS)
        wq_g = _perm_head_rows(wq[g * IQ:(g + 1) * IQ])
        wk_g = _perm_head_rows(wk[g * HEAD_DIM:(g + 1) * HEAD_DIM])
        wv_g = wv[g * HEAD_DIM:(g + 1) * HEAD_DIM]
        wqT = _pmajor(_bf16(wq_g.T))
        wkvT = _pmajor(_bf16(np.concatenate([wk_g.T, wv_g.T], axis=1)))
        woT = _pmajor(_bf16(wo[:, g * IQ:(g + 1) * IQ].T))
        in_maps.append({
            "xT": np.ascontiguousarray(
                _pmajor(_bf16(x[b].T)).reshape(128, 16, 4, 512)
                .transpose(2, 0, 1, 3)),
            "wqT": wqT,
            "wkvT": wkvT,
            "woT": woT,
            "t1": _pmajor(_bf16(t1)),
            "t2": _pmajor(_bf16(t2)),
            "masks": _bf16(masks.swapaxes(0, 1)),
        })
    return in_maps


_CACHED_NC = None


def _get_nc():
    global _CACHED_NC
    if _CACHED_NC is None:
        from concourse import bacc

        nc = bacc.Bacc("TRN2", target_bir_lowering=False, debug=False)
        build_attention_core(nc)
        nc.compile()
        _CACHED_NC = nc
    return _CACHED_NC


def kernel(x, freq_cis, wq, wk, wv, wo):
    from concourse.bass_utils import run_bass_kernel_spmd

    nc = _get_nc()
    in_maps = make_core_inputs(x, freq_cis, wq, wk, wv, wo)
    res = run_bass_kernel_spmd(nc, in_maps, list(range(8)))
    out = np.zeros((B, S, DIM), dtype=np.float32)
    for core in range(8):
        b = core // N_GROUPS
        out[b] += res.results[core]["out_partial"].astype(np.float32)
    return out
